# revision 26
# baseline (speedup 1.0000x reference)
"""MultiHeadLatentAttention on 8 Trainium2 NeuronCores.

Sharding: 2 batches x 4 head-groups (4 heads each) = 8 cores.
Each core computes, for its batch b and heads [4*hg, 4*hg+4):
  q = x[b] @ Wq[:, cols]                  (computed transposed: qT [512, T])
  latent_new = LN(x[b] @ Wdown)           (transposed, replicated per batch)
  kT_all = (latent @ Wk[:, cols]).T       v_all = latent @ Wv[:, cols]
  scores.T, softmax (no max-subtraction; |scores| <= ~4), PV accumulation
  o_partial = attn_out @ Wo[rows, :]      -> [T, D] partial sum
Host sums the 4 partials per batch and stacks the 2 batches.

v2: all matmul operands in bf16 (fp32 PSUM accumulation) — enables fast
weight load, halves DMA/SBUF; weights + latent cache resident in SBUF;
exp batched over key-block pairs ([128,1024] per ACTIVATE) with score
matmuls software-pipelined one pair ahead; softmax denominator via
quad-summed ones-matmul chain emitted after each block loop.
"""

import numpy as np

N_HEADS = 16
T = 2048
D = 2048
LAT = 512
PAST = 2048
S = PAST + T  # 4096, below the 8192 cache cap
HD = D // N_HEADS  # 128
HPC = 4  # heads per core
LN_EPS = 1e-5
SCALE = 1.0 / float(np.sqrt(HD))
NJB = S // 128  # 32 key blocks
NTT = T // 512  # 4 query tiles
NDC = D // 128  # 16
NLC = LAT // 128  # 4
NPB = PAST // 128  # 16 past key blocks

_CACHE = {}


def _build():
    import concourse.bacc as bacc
    import concourse.mybir as mybir
    import concourse.tile as tile
    from concourse import bass_isa

    f32 = mybir.dt.float32
    bf16 = mybir.dt.bfloat16
    AF = mybir.ActivationFunctionType
    OP = mybir.AluOpType

    nc = bacc.Bacc("TRN2", target_bir_lowering=False, debug=False, num_devices=8)

    xT = nc.dram_tensor("xT", [D, T], bf16, kind="ExternalInput")
    xdT = nc.dram_tensor("xdT", [D, 512], bf16, kind="ExternalInput")
    lpT = nc.dram_tensor("lpT", [LAT, PAST], bf16, kind="ExternalInput")
    wq = nc.dram_tensor("wq", [D, LAT], bf16, kind="ExternalInput")
    wd = nc.dram_tensor("wd", [D, LAT], bf16, kind="ExternalInput")
    wk = nc.dram_tensor("wk", [LAT, LAT], bf16, kind="ExternalInput")
    wv = nc.dram_tensor("wv", [LAT, LAT], bf16, kind="ExternalInput")
    wo = nc.dram_tensor("wo", [LAT, D], bf16, kind="ExternalInput")
    g = nc.dram_tensor("g", [LAT], f32, kind="ExternalInput")
    b = nc.dram_tensor("bb", [LAT], f32, kind="ExternalInput")
    mask = nc.dram_tensor("mask", [128, 896], bf16, kind="ExternalInput")
    o = nc.dram_tensor("o", [T, D], bf16, kind="ExternalOutput")

    with tile.TileContext(nc) as tc:
        with (
            tc.tile_pool(name="consts", bufs=1) as consts,
            tc.tile_pool(name="persist", bufs=1) as persist,
        ):
            # mask[jj, c] = 1 iff c >= jj + 384, so cols [512, 640) are all
            # ones for every partition — doubles as the ones matrix for the
            # softmax-denominator matmul.
            mask_sb = consts.tile([128, 896], bf16)
            nc.sync.dma_start(out=mask_sb, in_=mask[:, :])
            ones_sb = mask_sb[:, 512:640]
            g_sb = consts.tile([128, NLC], f32)
            nc.sync.dma_start(out=g_sb, in_=g[:].rearrange("(lc p) -> p lc", p=128))
            b_sb = consts.tile([128, NLC], f32)
            nc.sync.dma_start(out=b_sb, in_=b[:].rearrange("(lc p) -> p lc", p=128))
            eps_sb = consts.tile([128, 1], f32)
            nc.vector.memset(eps_sb, LN_EPS)
            wk_sb = consts.tile([128, NLC, LAT], bf16)
            nc.sync.dma_start(
                out=wk_sb, in_=wk[:, :].rearrange("(lc p) n -> p lc n", p=128)
            )
            wv_sb = consts.tile([128, NLC, LAT], bf16)
            nc.sync.dma_start(
                out=wv_sb, in_=wv[:, :].rearrange("(lc p) n -> p lc n", p=128)
            )

            qT_sb = persist.tile([128, HPC, T], bf16)  # q transposed, per head
            latn_sb = persist.tile([128, NLC, T], bf16)  # new latent, transposed
            ao_sb = persist.tile([128, HPC, T], bf16)  # attn out transposed
            lp_sb = persist.tile([128, NLC, PAST], bf16)  # past latent cache

            # ---- Phase A: down-projection of this core's own T-quarter +
            # LayerNorm + 4-way AllGather of latent quarters (hidden under
            # the q projection), then the q projection itself.
            with (
                tc.tile_pool(name="wA", bufs=1) as wA,
                tc.tile_pool(name="xa", bufs=1) as xa,
                tc.tile_pool(name="dnp", bufs=1) as dnp,
                tc.tile_pool(name="stats", bufs=1) as stats,
                tc.tile_pool(name="dram", bufs=1, space="DRAM") as dram,
                tc.tile_pool(name="psA", bufs=1, space="PSUM") as psA,
            ):
                wq_sb = wA.tile([128, NDC, LAT], bf16)
                wd_sb = wA.tile([128, NDC, LAT], bf16)
                xd_sb = wA.tile([128, NDC, 512], bf16)

                def w_dma(sb, w, ch):
                    nc.sync.dma_start(
                        out=sb[:, ch * 4 : (ch + 1) * 4, :],
                        in_=w[ch * 512 : (ch + 1) * 512, :].rearrange(
                            "(dc p) n -> p dc n", p=128
                        ),
                    )

                def x_dma(dc, tp):
                    xt = xa.tile([128, 1024], bf16, tag=f"x{dc}", name=f"x{dc}")
                    nc.sync.dma_start(
                        out=xt,
                        in_=xT[dc * 128 : (dc + 1) * 128, tp * 1024 : (tp + 1) * 1024],
                    )
                    return xt

                # DMA order: wd/xd chunks first so the down-projection starts
                # within ~2 us, then wq and the q-projection x tiles.
                for ch in range(4):
                    w_dma(wd_sb, wd, ch)
                    w_dma(xd_sb, xdT, ch)
                for ch in range(4):
                    w_dma(wq_sb, wq, ch)
                xts0 = [x_dma(dc, 0) for dc in range(NDC)]
                nc.sync.dma_start(
                    out=lp_sb, in_=lpT[:, :].rearrange("(lc p) t -> p lc t", p=128)
                )

                d_ps = [
                    psA.tile([128, 512], f32, tag=f"d{lc}", name=f"d_ps{lc}")
                    for lc in range(NLC)
                ]
                for dc in range(NDC):
                    for lc in range(NLC):
                        nc.tensor.matmul(
                            d_ps[lc],
                            lhsT=wd_sb[:, dc, lc * 128 : (lc + 1) * 128],
                            rhs=xd_sb[:, dc, :],
                            start=(dc == 0),
                            stop=(dc == NDC - 1),
                        )
                down = [
                    dnp.tile([128, 512], f32, tag=f"dn{lc}", name=f"down{lc}")
                    for lc in range(NLC)
                ]
                for lc in range(NLC):
                    nc.vector.tensor_copy(down[lc], d_ps[lc])
                # LayerNorm stats over the 512 latent dims (4 tiles x 128
                # partitions). Pack [sum | sumsq] side by side; one gpsimd
                # cross-partition all-reduce produces both, replicated.
                ss2 = stats.tile([128, 1024], f32, tag="ss2", name="ss2")
                nc.vector.tensor_add(ss2[:, 0:512], down[0], down[1])
                nc.vector.tensor_add(ss2[:, 0:512], ss2[:, 0:512], down[2])
                nc.vector.tensor_add(ss2[:, 0:512], ss2[:, 0:512], down[3])
                sqa = stats.tile([128, 512], f32, tag="sqa", name="sqa")
                nc.scalar.square(sqa, down[0])
                sqb = stats.tile([128, 512], f32, tag="sqb", name="sqb")
                nc.vector.tensor_mul(sqb, down[1], down[1])
                nc.vector.tensor_add(ss2[:, 512:1024], sqa, sqb)
                sqa2 = stats.tile([128, 512], f32, tag="sqa", name="sqa2")
                nc.scalar.square(sqa2, down[2])
                nc.vector.tensor_add(ss2[:, 512:1024], ss2[:, 512:1024], sqa2)
                sqb2 = stats.tile([128, 512], f32, tag="sqb", name="sqb2")
                nc.vector.tensor_mul(sqb2, down[3], down[3])
                nc.vector.tensor_add(ss2[:, 512:1024], ss2[:, 512:1024], sqb2)
                pr2 = stats.tile([128, 1024], f32, tag="pr2", name="pr2")
                nc.gpsimd.partition_all_reduce(
                    pr2, ss2, channels=128, reduce_op=bass_isa.ReduceOp.add
                )
                mu = stats.tile([128, 512], f32, tag="mu", name="mu")
                nc.vector.tensor_scalar_mul(mu, pr2[:, 0:512], 1.0 / LAT)
                vtmp = stats.tile([128, 512], f32, tag="vtmp", name="vtmp")
                nc.vector.tensor_mul(vtmp, mu, mu)
                sd = stats.tile([128, 512], f32, tag="sd", name="sd")
                nc.vector.scalar_tensor_tensor(
                    out=sd,
                    in0=pr2[:, 512:1024],
                    scalar=1.0 / LAT,
                    in1=vtmp,
                    op0=OP.mult,
                    op1=OP.subtract,
                )
                nc.scalar.activation(sd, sd, AF.Sqrt, bias=eps_sb)
                rstd = stats.tile([128, 512], f32, tag="rstd", name="rstd")
                nc.vector.reciprocal_approx_fast(rstd, sd)
                latq = stats.tile([128, NLC, 512], bf16, tag="latq", name="latq")
                for lc in range(NLC):
                    t1 = stats.tile([128, 512], f32, tag="sqa", name="t1")
                    nc.vector.tensor_sub(t1, down[lc], mu)
                    t2 = stats.tile([128, 512], f32, tag="sqb", name="t2")
                    nc.vector.tensor_mul(t2, t1, rstd)
                    nc.vector.tensor_scalar(
                        latq[:, lc, :],
                        t2,
                        g_sb[:, lc : lc + 1],
                        b_sb[:, lc : lc + 1],
                        OP.mult,
                        OP.add,
                    )
                # share latent quarters within the batch's 4 cores
                in_b = dram.tile([128, NLC * 512], bf16)
                out_b = dram.tile([512, NLC * 512], bf16)
                nc.gpsimd.dma_start(in_b[:], latq.rearrange("p l n -> p (l n)"))
                nc.gpsimd.collective_compute(
                    "AllGather",
                    OP.bypass,
                    replica_groups=[[0, 1, 2, 3], [4, 5, 6, 7]],
                    ins=[in_b.opt()],
                    outs=[out_b.opt()],
                )
                for c in range(4):
                    nc.gpsimd.dma_start(
                        latn_sb[:, :, c * 512 : (c + 1) * 512],
                        out_b[c * 128 : (c + 1) * 128, :].rearrange(
                            "p (l n) -> p l n", l=NLC
                        ),
                    )

                # ---- q projection over all T
                for tp in range(2):  # 1024-wide x tiles, two 512 query tiles each
                    xts = xts0 if tp == 0 else [x_dma(dc, 1) for dc in range(NDC)]
                    for t2 in range(2):
                        tt = tp * 2 + t2
                        tsl = slice(tt * 512, (tt + 1) * 512)
                        xsl = slice(t2 * 512, (t2 + 1) * 512)
                        q_ps = [
                            psA.tile([128, 512], f32, tag=f"q{qc}", name=f"q_ps{qc}")
                            for qc in range(HPC)
                        ]
                        for dc in range(NDC):
                            for qc in range(HPC):
                                nc.tensor.matmul(
                                    q_ps[qc],
                                    lhsT=wq_sb[:, dc, qc * 128 : (qc + 1) * 128],
                                    rhs=xts[dc][:, xsl],
                                    start=(dc == 0),
                                    stop=(dc == NDC - 1),
                                )
                        for qc in range(HPC):
                            nc.vector.tensor_copy(qT_sb[:, qc, tsl], q_ps[qc])

            # ---- Phase B: k/v up-projection for all 4 heads, then
            # ---- Phase C: per-head attention
            with (
                tc.tile_pool(name="kv", bufs=1) as kv,
                tc.tile_pool(name="pp", bufs=1) as pp,
                tc.tile_pool(name="ctmp", bufs=2) as ctmp,
                tc.tile_pool(name="ost", bufs=1) as ost,
                tc.tile_pool(name="psC", bufs=1, space="PSUM") as psC,
            ):
                kT_all = kv.tile([128, HPC, S], bf16)  # [hd, head, keys]
                v_all = kv.tile([128, NJB, LAT], bf16)  # [keys, block, head*hd]

                def lat(lc, gs):
                    if gs < PAST // 512:
                        return lp_sb[:, lc, gs * 512 : (gs + 1) * 512]
                    gn = gs - PAST // 512
                    return latn_sb[:, lc, gn * 512 : (gn + 1) * 512]

                def gen_b(gs):
                    gsl = slice(gs * 512, (gs + 1) * 512)
                    for h in range(HPC):
                        k_ps = psC.tile(
                            [128, 512], f32, tag="ops", bufs=2, name="k_ps"
                        )
                        for lc in range(NLC):
                            nc.tensor.matmul(
                                k_ps,
                                lhsT=wk_sb[:, lc, h * 128 : (h + 1) * 128],
                                rhs=lat(lc, gs),
                                start=(lc == 0),
                                stop=(lc == NLC - 1),
                            )
                            yield
                        nc.vector.tensor_copy(kT_all[:, h, gsl], k_ps)
                    for j4 in range(4):
                        v_ps = psC.tile(
                            [128, 512], f32, tag="ops", bufs=2, name="v_ps"
                        )
                        for lc in range(NLC):
                            nc.tensor.matmul(
                                v_ps,
                                lhsT=lat(lc, gs)[:, j4 * 128 : (j4 + 1) * 128],
                                rhs=wv_sb[:, lc, :],
                                start=(lc == 0),
                                stop=(lc == NLC - 1),
                            )
                            yield
                        nc.vector.tensor_copy(v_all[:, gs * 4 + j4, :], v_ps)

                # key-groups 0-6 up front (tt=0 needs groups 0-4); group 7 is
                # interleaved into tt=0's attention, which only needs it at tt=3
                for gs in range(7):
                    for _ in gen_b(gs):
                        pass

                wo_sb = kv.tile([128, HPC, D], bf16)
                for dt_ in range(4):
                    nc.sync.dma_start(
                        out=wo_sb[:, :, dt_ * 512 : (dt_ + 1) * 512],
                        in_=wo[:, dt_ * 512 : (dt_ + 1) * 512].rearrange(
                            "(hc p) n -> p hc n", p=128
                        ),
                    )

                def gen_d(tt):
                    # output projection for query rows [tt*512, tt*512+512) —
                    # interleaved into the next tt's attention PE slack
                    for tc_ in range(4 * tt, 4 * tt + 4):
                        o_sb = ost.tile([128, D], bf16, tag="osb", bufs=2, name="o_sb")
                        for dt_ in range(4):
                            o_ps = psC.tile(
                                [128, 512], f32, tag="ops", bufs=2, name="o_ps"
                            )
                            for hc in range(HPC):
                                nc.tensor.matmul(
                                    o_ps,
                                    lhsT=ao_sb[:, hc, tc_ * 128 : (tc_ + 1) * 128],
                                    rhs=wo_sb[:, hc, dt_ * 512 : (dt_ + 1) * 512],
                                    start=(hc == 0),
                                    stop=(hc == HPC - 1),
                                )
                                yield
                            nc.vector.tensor_copy(
                                o_sb[:, dt_ * 512 : (dt_ + 1) * 512], o_ps
                            )
                        nc.sync.dma_start(
                            out=o[tc_ * 128 : (tc_ + 1) * 128, :], in_=o_sb
                        )

                gens = []

                def pump(n):
                    while n > 0 and gens:
                        try:
                            next(gens[0])
                            n -= 1
                        except StopIteration:
                            gens.pop(0)

                for tt in range(NTT):
                    if tt == 0:
                        gens.append(gen_b(7))
                    for h in range(HPC):
                        hsl = slice(h * 128, (h + 1) * 128)
                        tsl = slice(tt * 512, (tt + 1) * 512)
                        # fully visible blocks, then the 4 diagonal blocks;
                        # diagonal pairs carry query trims (rrA, rrB): block
                        # jbn (local r4) only reaches queries >= r4*128.
                        nfull = NPB + 4 * tt
                        d0 = nfull
                        pairs = [
                            (2 * i, 2 * i + 1, 0, 0) for i in range(nfull // 2)
                        ] + [(d0, d0 + 1, 0, 128), (d0 + 2, d0 + 3, 256, 384)]
                        np_ = len(pairs)
                        # attention accumulator and softmax denominator share
                        # one 2-bank psum tile (same lifetime)
                        ad = psC.tile([128, 1024], f32, tag="ad", bufs=1, name="ad")
                        attn_ps = ad[:, 0:512]
                        den_ps = ad[:, 512:1024]

                        def emit_s(i):
                            jbA, jbB, rA, rB = pairs[i]
                            sp = psC.tile(
                                [128, 1024], f32, tag="sps", bufs=2, name="s_ps"
                            )
                            nc.tensor.matmul(
                                sp[:, rA:512],
                                lhsT=kT_all[:, h, jbA * 128 : (jbA + 1) * 128],
                                rhs=qT_sb[:, h, tt * 512 + rA : (tt + 1) * 512],
                                start=True,
                                stop=True,
                            )
                            nc.tensor.matmul(
                                sp[:, 512 + rB : 1024],
                                lhsT=kT_all[:, h, jbB * 128 : (jbB + 1) * 128],
                                rhs=qT_sb[:, h, tt * 512 + rB : (tt + 1) * 512],
                                start=True,
                                stop=True,
                            )
                            return sp

                        sp_cur = emit_s(0)
                        pd_prev = None
                        for i in range(np_):
                            jbA, jbB, rA, rB = pairs[i]
                            p_sb = pp.tile(
                                [128, 1024], bf16, tag="p", bufs=3, name="p_sb"
                            )
                            # single exp over [rA:1024]: any gap between the
                            # two written score ranges holds stale-but-finite
                            # psum; the mask multiplies zero those p columns.
                            nc.scalar.activation(
                                p_sb[:, rA:1024], sp_cur[:, rA:1024],
                                AF.Exp, scale=SCALE,
                            )
                            if i + 1 < np_:
                                sp_cur = emit_s(i + 1)
                            for half, jb in ((0, jbA), (1, jbB)):
                                jbn = jb - NPB
                                if jbn >= 0 and jbn // 4 == tt:
                                    rr = (jbn % 4) * 128
                                    psl = slice(half * 512, half * 512 + 512)
                                    nc.vector.tensor_mul(
                                        p_sb[:, psl],
                                        p_sb[:, psl],
                                        mask_sb[:, 384 - rr : 896 - rr],
                                    )
                            nc.tensor.matmul(
                                attn_ps[:, rA:512],
                                lhsT=v_all[:, jbA, hsl],
                                rhs=p_sb[:, rA:512],
                                start=(i == 0),
                                stop=False,
                            )
                            nc.tensor.matmul(
                                attn_ps[:, rB:512],
                                lhsT=v_all[:, jbB, hsl],
                                rhs=p_sb[:, 512 + rB : 1024],
                                start=False,
                                stop=(i == np_ - 1),
                            )
                            pd = pp.tile([128, 512], bf16, tag="pd", bufs=2, name="pd")
                            nc.vector.tensor_add(pd, p_sb[:, 0:512], p_sb[:, 512:1024])
                            if i % 2 == 0:
                                pd_prev = pd
                            else:
                                pd2 = pp.tile(
                                    [128, 512], bf16, tag="pd2", bufs=4, name="pd2"
                                )
                                nc.vector.tensor_add(pd2, pd_prev, pd)
                                nc.tensor.matmul(
                                    den_ps,
                                    lhsT=ones_sb,
                                    rhs=pd2,
                                    start=(i == 1),
                                    stop=(i == np_ - 1),
                                )
                            pump(1)
                        rec = ctmp.tile([128, 512], f32, tag="rec", name="rec")
                        nc.vector.reciprocal_approx_fast(rec, den_ps)
                        nc.vector.tensor_mul(ao_sb[:, h, tsl], attn_ps, rec)
                        # cover the division latency (ad bufs=1) with
                        # interleaved matmuls before the next head starts
                        pump(4)
                    gens.append(gen_d(tt))
                pump(1 << 30)

    nc.compile()
    return nc


def _get_nc():
    if "nc" not in _CACHE:
        _CACHE["nc"] = _build()
    return _CACHE["nc"]


def _make_mask():
    # B[jj, c] = 1.0 iff c >= jj + 384; sliced at 384-r it gives the
    # causal staircase "visible iff i >= jj + r" for r in {0,128,256,384}.
    jj = np.arange(128)[:, None]
    cc = np.arange(896)[None, :]
    return cc >= jj + 384


def _in_maps(x, latent_prev, Wq, Wdown, Wk_up, Wv_up, ln_g, ln_b, Wo):
    import ml_dtypes

    bf = ml_dtypes.bfloat16
    fb = lambda a: np.ascontiguousarray(np.asarray(a, dtype=np.float32).astype(bf))
    f = lambda a: np.ascontiguousarray(np.asarray(a, dtype=np.float32))
    mask = _make_mask().astype(bf)
    wd_b = fb(Wdown)
    g_ = f(ln_g)
    b_ = f(ln_b)
    maps = []
    for bi in range(2):
        xTb = fb(np.asarray(x)[bi].T)
        lpTb = fb(np.asarray(latent_prev)[bi].T)
        for hg in range(4):
            sl = slice(hg * 512, (hg + 1) * 512)
            maps.append(
                {
                    "xT": xTb,
                    "xdT": np.ascontiguousarray(xTb[:, sl]),
                    "lpT": lpTb,
                    "wq": fb(np.asarray(Wq)[:, sl]),
                    "wd": wd_b,
                    "wk": fb(np.asarray(Wk_up)[:, sl]),
                    "wv": fb(np.asarray(Wv_up)[:, sl]),
                    "wo": fb(np.asarray(Wo)[sl, :]),
                    "g": g_,
                    "bb": b_,
                    "mask": mask,
                }
            )
    return maps


def run(trace=False, **inputs):
    from concourse.bass_utils import run_bass_kernel_spmd

    nc = _get_nc()
    maps = _in_maps(**inputs)
    res = run_bass_kernel_spmd(nc, maps, core_ids=list(range(8)), trace=trace)
    outs = [res.results[c]["o"].astype(np.float32) for c in range(8)]
    out = np.stack(
        [
            outs[0] + outs[1] + outs[2] + outs[3],
            outs[4] + outs[5] + outs[6] + outs[7],
        ],
        axis=0,
    )
    return out, res


def kernel(**inputs):
    out, _ = run(trace=False, **inputs)
    return out
